# revision 1
# baseline (speedup 1.0000x reference)
"""FJSP decoder kernel for Trainium2, data-parallel over batch on 8 NeuronCores.

Key algebraic restructuring: q/k/v for the flattened (job, machine) pair
s=(j,m) decompose as x[s] = xj[j] + xm[m], so the joint-axis attention
softmax factorizes exactly:

  score[s, (j',m')] = E[s,j'] + F[s,m']      (E from A,C; F from B,Dm)
  softmax_t(score) @ v = softmax_j'(E) @ vj + softmax_m'(F) @ vm

and with E[(j,m),j'] = (A[j,j'] + C[m,j'])/sqrt(QD) the row softmax of E
itself factorizes through exp(A)*exp(C), giving per head only J*J-sized
matmuls -- the [S,S] = [2000,2000] score matrix is never materialized.
The multi-head combine collapses through w2 = Wmhc @ Wshc into per-head
scalars uv = v @ w2, so the whole decoder reduces to [100,20]-shaped work:

  SE|Nj = eAT.T @ [eCT | eCT*uvj];  SF|Nm = eBT.T @ [eDT | eDT*uvm]
  score1 = (sum_h Nj/SE + Nm/SF + bias)/sqrt(D)
  p = softmax_flat(10*tanh(score1) + mask)   (tanh via exp, one ACT table)

Layout notes: heads are padded to 32-partition strips (two groups of 4
heads) so per-head K=16 contractions become legal K=32 matmuls at base
partitions {0,32,64,96}; k/v projection tiles carry 80 zero columns so
every per-head matmul runs in the same (32-row, 128-col) PE tiling mode.
All inputs are host-packed into two DRAM tensors (weights, activations)
so the kernel issues exactly two input DMAs.
"""

import math

import numpy as np

import concourse.bass as bass
import concourse.mybir as mybir
import concourse.tile as tile
from concourse.bass_utils import run_bass_kernel_spmd
from concourse.masks import make_identity

F32 = mybir.dt.float32
AF = mybir.ActivationFunctionType
OP = mybir.AluOpType
AX = mybir.AxisListType

D, H, QD = 128, 8, 16
B, J, M = 8, 100, 20
HQ = H * QD  # 128
INV_SQ = 1.0 / math.sqrt(QD)  # 0.25
SD = math.sqrt(D)

# edata column layout: [ej 0:128 | em 128:256 | mask 256:276 | smallw 276:279]
EJ0, EM0, MK0, SW0 = 0, 128, 256, 276
EDATA_W = 279

# ---------------------------------------------------------------------------
# gen3 walrus accepts one sync-wait per instruction. Tile's kernel-tail
# drain accumulates one wait per active logical processor on a single
# Drain: spread them across engines (parallel waiting). Tile's semaphore
# pass can also attach >1 wait to ordinary instructions: shed extras onto
# same-engine NoOps inserted right before the offender.
_PATCHED = False


def _install_drain_patch():
    global _PATCHED
    if _PATCHED:
        return
    from concourse.tile import ScopedClock, TileContext

    def _split_drain_and_barrier(self, tick_clock, wait_clock):
        drain_inst = self.nc.sync.drain()
        wait_clock.add_sem_waits(
            drain_inst.ins, ScopedClock({None: tick_clock.global_clock})
        )
        si = drain_inst.ins.sync_info
        waits = list(si.on_wait) if si is not None else []
        if len(waits) > 1:
            assert not si.on_update
            sems = {s.name: s for s in self.sems.allocated().values()}
            drain_inst.ins.sync_info = None
            drain_inst.wait_op(sems[waits[0].ant_name], waits[0].wait_value, "sem-ge")
            engines = [
                self.nc.scalar,
                self.nc.vector,
                self.nc.tensor,
                self.nc.gpsimd,
                self.nc.sync,
            ]
            for i, w in enumerate(waits[1:]):
                extra = engines[i % len(engines)].drain()
                extra.wait_op(sems[w.ant_name], w.wait_value, "sem-ge")
        self.nc.all_engine_barrier()
        assert self.sems is not None
        popped = self.nc._tile_sem_poison_stack.pop()
        assert popped is self._sem_poison
        self.nc.clear_and_free_semaphores(list(self.sems.allocated().values()))

    TileContext._drain_and_barrier = _split_drain_and_barrier
    _PATCHED = True


def _split_multi_waits(nc):
    import bass_rust

    ctr = 0
    for fn in nc.m.functions:
        for bb in fn.blocks:
            il = bb.instructions
            if not any(
                i.sync_info is not None and len(i.sync_info.on_wait) > 1 for i in il
            ):
                continue
            new = []
            for ins in il:
                si = ins.sync_info
                if si is not None and len(si.on_wait) > 1:
                    waits = list(si.on_wait)
                    ups = list(si.on_update)
                    for w in waits[:-1]:
                        nop = mybir.InstNoOp(name=f"I-waitsplit-{ctr}", ins=[], outs=[])
                        ctr += 1
                        nop.engine = ins.engine
                        nop.sync_info = bass_rust.SyncInfo(on_update=[], on_wait=[w])
                        new.append(nop)
                    ins.sync_info = bass_rust.SyncInfo(
                        on_update=ups, on_wait=[waits[-1]]
                    )
                new.append(ins)
            bb.instructions = new


def _chunk2(ap_slice, chunk_step):
    """Matmul rhs built from two equal column chunks `chunk_step` apart."""
    return bass.AP(
        tensor=ap_slice.tensor,
        offset=ap_slice.offset,
        ap=[ap_slice.ap[0], [chunk_step, 2], ap_slice.ap[1]],
    )


def _build():
    nc = bass.Bass()
    # wqkv[:, i, :]: 0=Wq3-job 1=Wq3-mach 2=Wk-job 3=Wk-mach 4=Wv-job
    # 5=Wv-mach 6=Wmhc
    wqkv_d = nc.dram_tensor("wqkv", [D, 7, D], F32, kind="ExternalInput")
    ed_d = nc.dram_tensor("edata", [D, EDATA_W], F32, kind="ExternalInput")
    out_d = nc.dram_tensor("out", [J, M], F32, kind="ExternalOutput")

    with tile.TileContext(nc) as tc:
        with (
            tc.tile_pool(name="persist", bufs=1) as pp,
            tc.tile_pool(name="rot", bufs=8) as rp,
            tc.tile_pool(name="ps_big", bufs=2, space="PSUM") as ps_big,
            tc.tile_pool(name="ps_s1", bufs=6, space="PSUM") as ps_s1,
        ):
            # ---- constants that gate the PE transposes ------------------
            ident = pp.tile([D, D], F32, tag="ident")
            make_identity(nc, ident)

            # ---- the two input DMAs -------------------------------------
            ed_sb = pp.tile([D, EDATA_W], F32, tag="edata")
            nc.sync.dma_start(out=ed_sb, in_=ed_d[:])
            wqkv_sb = pp.tile([D, 7, D], F32, tag="wqkv")
            nc.sync.dma_start(out=wqkv_sb, in_=wqkv_d[:])

            ej_v = ed_sb[0:J, EJ0 : EJ0 + D]
            em_v = ed_sb[0:M, EM0 : EM0 + D]
            mask_v = ed_sb[0:J, MK0 : MK0 + M]
            bmhc_v = ed_sb[:, SW0 : SW0 + 1]
            wshc_v = ed_sb[:, SW0 + 1 : SW0 + 2]
            bshc_v = ed_sb[0:1, SW0 + 2 : SW0 + 3]

            # k/v projection tiles get 80 zero cols (120:200) so machine-
            # side per-head matmuls run with M=100 (128-col PE mode)
            pT_sb = {}
            for nm in ("q", "k", "v"):
                for grp in range(2):
                    w = 120 if nm == "q" else 200
                    sb = pp.tile([D, w], F32, tag=f"{nm}T{grp}")
                    if nm != "q":
                        nc.gpsimd.memset(sb[:, 120:200], 0.0)
                    pT_sb[(nm, grp)] = sb

            ones_sb = pp.tile([D, D], F32, tag="ones")
            nc.gpsimd.memset(ones_sb, 1.0)

            # padded weights: head h -> 32-strip 32g..32g+16 (g = h%4) in
            # group A (h<4) / B (h>=4); the other 16 lanes zero.
            wpad = pp.tile([D, 12, D], F32, tag="wpad")
            wpad_idx = {}
            idx = 0
            for nm_i, nm in enumerate(("q", "k", "v")):
                for half in range(2):
                    for grp in range(2):
                        wpad_idx[(nm, half, grp)] = idx
                        eng = nc.vector if nm == "q" else nc.gpsimd
                        tv = wpad[:, idx, :].rearrange("p (g c) -> p g c", c=32)
                        eng.memset(tv[:, :, 16:32], 0.0)
                        src = wqkv_sb[
                            :, nm_i * 2 + half, grp * 64 : (grp + 1) * 64
                        ].rearrange("p (g c) -> p g c", c=16)
                        eng.tensor_copy(out=tv[:, :, 0:16], in_=src)
                        idx += 1

            # ---- PE transposes (wmhcT first: longest downstream chain) --
            wmhcT_ps = ps_big.tile([D, 320], F32, tag="big")
            nc.tensor.transpose(wmhcT_ps[:, 0:HQ], wqkv_sb[:, 6, :], ident)
            wmhcT_sb = pp.tile([D, HQ], F32, tag="wmhcT")
            nc.scalar.copy(out=wmhcT_sb, in_=wmhcT_ps[:, 0:HQ])

            ejT_ps = ps_big.tile([D, 320], F32, tag="big")
            nc.tensor.transpose(ejT_ps[:, 0:J], ej_v, ident[0:J, 0:J])
            ejT_sb = pp.tile([D, J], F32, tag="ejT")
            nc.scalar.copy(out=ejT_sb, in_=ejT_ps[:, 0:J])

            emT_ps = ps_big.tile([D, 320], F32, tag="big")
            nc.tensor.transpose(emT_ps[:, 0:M], em_v, ident[0:M, 0:M])
            emT_sb = pp.tile([D, M], F32, tag="emT")
            nc.scalar.copy(out=emT_sb, in_=emT_ps[:, 0:M])

            # WmhcT with columns in padded-head layout, per group
            wmhcPT = pp.tile([D, 2, D], F32, tag="wmhcPT")
            for grp in range(2):
                tv = wmhcPT[:, grp, :].rearrange("p (g c) -> p g c", c=32)
                nc.gpsimd.memset(tv[:, :, 16:32], 0.0)
                src = wmhcT_sb[:, grp * 64 : (grp + 1) * 64].rearrange(
                    "p (g c) -> p g c", c=16
                )
                nc.gpsimd.tensor_copy(out=tv[:, :, 0:16], in_=src)

            # ---- 128x128 mode: projections, w2pad, bias -----------------
            for nm in ("v", "k", "q"):
                for grp in range(2):
                    ps = ps_big.tile([D, 320], F32, tag="big")
                    nc.tensor.matmul(
                        out=ps[:, 0:J],
                        lhsT=wpad[:, wpad_idx[(nm, 0, grp)], :],
                        rhs=ejT_sb,
                    )
                    nc.tensor.matmul(
                        out=ps[:, J : J + M],
                        lhsT=wpad[:, wpad_idx[(nm, 1, grp)], :],
                        rhs=emT_sb,
                    )
                    sb = pT_sb[(nm, grp)]
                    if nm == "v":
                        nc.vector.tensor_copy(out=sb[:, 0:120], in_=ps[:, 0:120])
                    else:
                        nc.scalar.copy(out=sb[:, 0:120], in_=ps[:, 0:120])

            w2pad_sb = []
            for grp in range(2):
                ps = ps_big.tile([D, 320], F32, tag="big")
                nc.tensor.matmul(
                    out=ps[:, 0:1], lhsT=wmhcPT[:, grp, :], rhs=wshc_v
                )
                sb = pp.tile([D, 1], F32, tag=f"w2pad{grp}")
                nc.vector.tensor_copy(out=sb, in_=ps[:, 0:1])
                w2pad_sb.append(sb)

            # bias_const = b_mhc @ Wshc + b_shc, broadcast over J partitions
            bw = pp.tile([D, 1], F32, tag="bw")
            nc.vector.tensor_mul(out=bw, in0=bmhc_v, in1=wshc_v)
            nc.vector.tensor_add(out=bw[0:1, 0:1], in0=bw[0:1, 0:1], in1=bshc_v)
            bias_ps = ps_big.tile([D, 320], F32, tag="big")
            nc.tensor.matmul(out=bias_ps[0:J, 0:1], lhsT=ones_sb[:, 0:J], rhs=bw)
            biasb = pp.tile([J, 1], F32, tag="biasb")
            nc.scalar.mul(out=biasb, in_=bias_ps[0:J, 0:1], mul=2.0 / SD)

            # ---- (32,128) mode: uv vectors + per-head products ----------
            uvj_ps = ps_big.tile([D, 320], F32, tag="big")
            uvm_ps = ps_big.tile([D, 320], F32, tag="big")
            for h in range(H):
                grp, g = divmod(h, 4)
                vt = pT_sb[("v", grp)]
                nc.tensor.matmul(
                    out=uvj_ps[0:J, h : h + 1],
                    lhsT=vt[32 * g : 32 * g + 32, 0:J],
                    rhs=w2pad_sb[grp][32 * g : 32 * g + 32, :],
                    tile_position=(32 * g, 0),
                )
                nc.tensor.matmul(
                    out=uvm_ps[0:J, h : h + 1],
                    lhsT=vt[32 * g : 32 * g + 32, 100:200],
                    rhs=w2pad_sb[grp][32 * g : 32 * g + 32, :],
                    tile_position=(32 * g, 0),
                )
            uvj_sb = pp.tile([J, H], F32, tag="uvj")
            nc.vector.tensor_copy(out=uvj_sb, in_=uvj_ps[0:J, 0:H])
            uvm_sb = pp.tile([M, H], F32, tag="uvm")
            nc.vector.tensor_copy(out=uvm_sb, in_=uvm_ps[0:M, 0:H])

            # per head: o_ps = [AT|CT | BT|DT(+zeros)], one exp, uv scales
            f_ps = ps_big.tile([D, 8, 40], F32, tag="big")
            s_ps = ps_big.tile([D, 8, 40], F32, tag="big")
            eE = []
            for h in range(H):
                grp, g = divmod(h, 4)
                kt, qt = pT_sb[("k", grp)], pT_sb[("q", grp)]
                ps = ps_s1.tile([D, 240], F32, tag="s1")
                nc.tensor.matmul(
                    out=ps[0:J, 0:120],
                    lhsT=kt[32 * g : 32 * g + 32, 0:J],
                    rhs=qt[32 * g : 32 * g + 32, 0:120],
                    tile_position=(32 * g, 0),
                )
                nc.tensor.matmul(
                    out=ps[0:J, 120:240],
                    lhsT=kt[32 * g : 32 * g + 32, 100:200],
                    rhs=qt[32 * g : 32 * g + 32, 0:120],
                    tile_position=(32 * g, 0),
                )
                e1 = rp.tile([D, 280], F32, tag="eE")
                nc.scalar.activation(
                    out=e1[0:J, 0:240], in_=ps[0:J, 0:240], func=AF.Exp, scale=INV_SQ
                )
                nc.vector.tensor_scalar_mul(
                    out=e1[0:J, 240:260],
                    in0=e1[0:J, 100:120],
                    scalar1=uvj_sb[:, h : h + 1],
                )
                nc.vector.tensor_scalar_mul(
                    out=e1[0:M, 260:280],
                    in0=e1[0:M, 220:240],
                    scalar1=uvm_sb[:, h : h + 1],
                )
                eE.append(e1)
                # mm4: [SF|Nm] = eBT.T @ [eDT | eDT*uvm]   (K=20)
                nc.tensor.matmul(
                    out=f_ps[0:J, h, :],
                    lhsT=e1[0:M, 120:220],
                    rhs=_chunk2(e1[0:M, 220:240], 40),
                )
                # mm3: [SE|Nj] = eAT.T @ [eCT | eCT*uvj]   (K=100)
                nc.tensor.matmul(
                    out=s_ps[0:J, h, :],
                    lhsT=e1[0:J, 0:J],
                    rhs=_chunk2(e1[0:J, 100:120], 140),
                )

            def pmh(ap3):  # [p, h, m] -> [p, m, h]
                return ap3.rearrange("p h m -> p m h")

            # F-side combine first
            rF = pp.tile([J, M, H], F32, tag="rF")
            nc.vector.reciprocal(out=rF, in_=pmh(f_ps[0:J, :, 0:M]))
            d2 = pp.tile([J, M, H], F32, tag="d2")
            nc.vector.tensor_mul(out=d2, in0=pmh(f_ps[0:J, :, M : 2 * M]), in1=rF)

            # ---- combine: sum_h Nj/SE + Nm/SF ---------------------------
            rE = pp.tile([J, M, H], F32, tag="rE")
            nc.vector.reciprocal(out=rE, in_=pmh(s_ps[0:J, :, 0:M]))
            c8 = pp.tile([J, M, H], F32, tag="c8")
            nc.vector.scalar_tensor_tensor(
                out=c8, in0=pmh(s_ps[0:J, :, M : 2 * M]), scalar=1.0, in1=rE,
                op0=OP.mult, op1=OP.mult,
            )
            nc.vector.tensor_add(out=c8, in0=c8, in1=d2)
            c1 = pp.tile([J, M], F32, tag="c1")
            nc.vector.reduce_sum(out=c1, in_=c8, axis=AX.X)

            # tanh chain via exp (no ACT table switch):
            # logits ~ mask - 20/(exp(2*(c1+bias)/sqrt(D)) + 1)  (+const)
            u = pp.tile([J, M], F32, tag="u")
            nc.scalar.activation(out=u, in_=c1, func=AF.Exp, scale=2.0 / SD, bias=biasb)
            t1 = pp.tile([J, M], F32, tag="t1")
            nc.scalar.add(out=t1, in_=u, add=1.0)
            r = pp.tile([J, M], F32, tag="r")
            nc.vector.reciprocal(out=r, in_=t1)
            arg = pp.tile([J, M], F32, tag="arg")
            nc.vector.scalar_tensor_tensor(
                out=arg, in0=r, scalar=-20.0, in1=mask_v, op0=OP.mult, op1=OP.add
            )
            e_sb = pp.tile([J, M], F32, tag="e")
            s_row = pp.tile([J, 1], F32, tag="srow")
            nc.scalar.activation(
                out=e_sb, in_=arg, func=AF.Exp, scale=1.0, accum_out=s_row
            )
            totb_ps = ps_big.tile([D, 320], F32, tag="big")
            nc.tensor.matmul(out=totb_ps[0:J, 0:1], lhsT=ones_sb[0:J, 0:J], rhs=s_row)
            rtot = pp.tile([J, 1], F32, tag="rtot")
            nc.vector.reciprocal(out=rtot, in_=totb_ps[0:J, 0:1])
            out_t = pp.tile([J, M], F32, tag="outt")
            nc.vector.tensor_scalar_mul(out=out_t, in0=e_sb, scalar1=rtot)
            nc.sync.dma_start(out=out_d[:], in_=out_t)

    _split_multi_waits(nc)
    return nc


_NC = None
last_results = None


def kernel(**inputs):
    global _NC, last_results
    _install_drain_patch()
    if _NC is None:
        _NC = _build()

    wqkv = np.empty((D, 7, D), np.float32)
    for i, nm in enumerate(("Wq3", "Wk", "Wv")):
        w = np.asarray(inputs[nm], np.float32)
        wqkv[:, 2 * i, :] = w[:D]
        wqkv[:, 2 * i + 1, :] = w[D:]
    wqkv[:, 6, :] = np.asarray(inputs["Wmhc"], np.float32)

    ed_base = np.zeros((D, EDATA_W), np.float32)
    ed_base[:, SW0] = np.asarray(inputs["b_mhc"], np.float32).reshape(D)
    ed_base[:, SW0 + 1] = np.asarray(inputs["Wshc"], np.float32).reshape(D)
    ed_base[0, SW0 + 2] = np.float32(np.asarray(inputs["b_shc"]).reshape(-1)[0])

    ejs = np.asarray(inputs["encoded_job"], np.float32)
    ems = np.asarray(inputs["encoded_machine"], np.float32)
    msks = np.asarray(inputs["ninf_mask"], np.float32)

    in_maps = []
    for b in range(B):
        ed = ed_base.copy()
        ed[0:J, EJ0 : EJ0 + D] = ejs[b]
        ed[0:M, EM0 : EM0 + D] = ems[b]
        ed[0:J, MK0 : MK0 + M] = msks[b]
        in_maps.append({"wqkv": wqkv, "edata": ed})

    last_results = run_bass_kernel_spmd(_NC, in_maps, core_ids=list(range(B)))
    out = np.stack(
        [last_results.results[b]["out"].reshape(J * M) for b in range(B)]
    )
    return out.astype(np.float32)



# revision 4
# speedup vs baseline: 1.5478x; 1.5478x over previous
"""FJSP decoder kernel for Trainium2, data-parallel over batch on 8 NeuronCores.

Factorized attention (see derivation in comments): q/k/v of the flattened
(job, machine) pair s=(j,m) split as x[s] = xj[j] + xm[m], so the joint
softmax over t=(j',m') factorizes exactly:

  exp(score[s,t]) = expE[s,j'] * expF[s,m']
  softmax_t(score) @ v . w2 = Nj/SE + Nm/SF      (per head)

with expE[(j,m),j'] = eA[j,j']*eC[m,j'], expF[(j,m),m'] = eB[j,m']*eD[m,m'].
The multi-head combine collapses through w2 = Wmhc @ Wshc, so v only enters
via uv = x @ (Wv_blocks @ w2) -- the v projection never runs on device.

Device-side layout: per head h (grp=h//4, strip g=h%4) one joint matmul with
stationary [kjT | kmT] ([32, 120]) against rhs [qjT | qmT] gives the full
[120, 120] block (rows 0:100 = A^T,C^T; rows 100:120 = B^T,D^T); one exp per
4-head group covers everything.  E-side contraction (K=100) reads the exp
tile in place via a 2-chunk AP; the F-side (K=120) uses a zero-framed rhs so
the B^T rows land in the same matmul.  All matmul operands are bf16 (4x PE
throughput vs f32); final softmax chain stays f32.

Host-side prep is layout/weight-folding only: weights pre-padded into the
32-strip head layout, activations pre-transposed, w2/uw folded.  One input
DMA, one output DMA.
"""

import math

import numpy as np
import ml_dtypes

import concourse.bass as bass
import concourse.mybir as mybir
import concourse.tile as tile
from concourse.bass_utils import run_bass_kernel_spmd

F32 = mybir.dt.float32
BF16 = mybir.dt.bfloat16
AF = mybir.ActivationFunctionType
OP = mybir.AluOpType
AX = mybir.AxisListType

D, H, QD = 128, 8, 16
B, J, M = 8, 100, 20
INV_SQ = 1.0 / math.sqrt(QD)  # 0.25
SD = math.sqrt(D)

# input column layout (all bf16, [128, NCOL])
EJ = 0                      # ejT [0:128, 0:100]
EM = 100                    # emT [0:128, 100:120]
WBLK = 120                  # 8 weight blocks of 128 cols each:
#   order: (k,j,g0) (k,m,g0) (k,j,g1) (k,m,g1) (q,j,g0) (q,m,g0) (q,j,g1) (q,m,g1)
UWJ = WBLK + 8 * 128        # 1144: uwj [0:128, 8]
UWM = UWJ + 8               # 1152: uwm [0:128, 8]
MK = UWM + 8                # 1160: mask [0:100, 20]
BC = MK + 20                # 1180: bias col [0:100, 1] = bias_c / sqrt(D)
NCOL = BC + 1               # 1181

# set False if HW rejects divide alu ops (falls back to reciprocal+mul)
USE_DIVIDE = True


# ---------------------------------------------------------------------------
# gen3 walrus accepts one sync-wait per instruction. Tile's kernel-tail
# drain accumulates one wait per active logical processor on a single
# Drain: spread them across engines (parallel waiting). Tile's semaphore
# pass can also attach >1 wait to ordinary instructions: shed extras onto
# same-engine NoOps inserted right before the offender.
_PATCHED = False


def _install_drain_patch():
    global _PATCHED
    if _PATCHED:
        return
    from concourse.tile import ScopedClock, TileContext

    def _split_drain_and_barrier(self, tick_clock, wait_clock):
        drain_inst = self.nc.sync.drain()
        wait_clock.add_sem_waits(
            drain_inst.ins, ScopedClock({None: tick_clock.global_clock})
        )
        si = drain_inst.ins.sync_info
        waits = list(si.on_wait) if si is not None else []
        if len(waits) > 1:
            assert not si.on_update
            sems = {s.name: s for s in self.sems.allocated().values()}
            drain_inst.ins.sync_info = None
            drain_inst.wait_op(sems[waits[0].ant_name], waits[0].wait_value, "sem-ge")
            engines = [
                self.nc.scalar,
                self.nc.vector,
                self.nc.tensor,
                self.nc.gpsimd,
                self.nc.sync,
            ]
            for i, w in enumerate(waits[1:]):
                extra = engines[i % len(engines)].drain()
                extra.wait_op(sems[w.ant_name], w.wait_value, "sem-ge")
        self.nc.all_engine_barrier()
        assert self.sems is not None
        popped = self.nc._tile_sem_poison_stack.pop()
        assert popped is self._sem_poison
        self.nc.clear_and_free_semaphores(list(self.sems.allocated().values()))

    TileContext._drain_and_barrier = _split_drain_and_barrier
    _PATCHED = True


def _split_multi_waits(nc):
    import bass_rust

    ctr = 0
    for fn in nc.m.functions:
        for bb in fn.blocks:
            il = bb.instructions
            if not any(
                i.sync_info is not None and len(i.sync_info.on_wait) > 1 for i in il
            ):
                continue
            new = []
            for ins in il:
                si = ins.sync_info
                if si is not None and len(si.on_wait) > 1:
                    waits = list(si.on_wait)
                    ups = list(si.on_update)
                    for w in waits[:-1]:
                        nop = mybir.InstNoOp(name=f"I-waitsplit-{ctr}", ins=[], outs=[])
                        ctr += 1
                        nop.engine = ins.engine
                        nop.sync_info = bass_rust.SyncInfo(on_update=[], on_wait=[w])
                        new.append(nop)
                    ins.sync_info = bass_rust.SyncInfo(
                        on_update=ups, on_wait=[waits[-1]]
                    )
                new.append(ins)
            bb.instructions = new


def _chunk2(ap_slice, chunk_step):
    """Matmul rhs built from two equal column chunks `chunk_step` apart."""
    return bass.AP(
        tensor=ap_slice.tensor,
        offset=ap_slice.offset,
        ap=[ap_slice.ap[0], [chunk_step, 2], ap_slice.ap[1]],
    )


def _build():
    nc = bass.Bass()
    inp_d = nc.dram_tensor("inp", [D, NCOL], BF16, kind="ExternalInput")
    out_d = nc.dram_tensor("out", [J, M], F32, kind="ExternalOutput")

    with tile.TileContext(nc) as tc:
        with (
            tc.tile_pool(name="persist", bufs=1) as pp,
            tc.tile_pool(name="eero", bufs=2) as rp,
            tc.tile_pool(name="ps_proj", bufs=2, space="PSUM") as ps_proj,
            tc.tile_pool(name="ps_att", bufs=2, space="PSUM") as ps_att,
            tc.tile_pool(name="ps_out", bufs=1, space="PSUM") as ps_out,
        ):
            # ---- single input DMA, issued first ------------------------
            inp_sb = pp.tile([D, NCOL], BF16, tag="inp")
            nc.sync.dma_start(out=inp_sb, in_=inp_d[:])

            # ---- constants (no input dependency; overlap the DMA) ------
            ones = pp.tile([J, J], F32, tag="ones")
            nc.gpsimd.memset(ones, 1.0)
            rz = []
            for h in range(H):
                t = pp.tile([120, 40], BF16, tag=f"rz{h}")
                nc.gpsimd.memset(t[0:100, :], 0.0)
                rz.append(t)

            ejT = inp_sb[:, EJ : EJ + J]
            emT = inp_sb[:, EM : EM + M]

            # exp(mask): off the critical path, folds the mask add into the
            # final softmax as a multiply
            expmask = pp.tile([J, M], F32, tag="expmask")
            nc.scalar.activation(
                out=expmask, in_=inp_sb[0:J, MK : MK + M], func=AF.Exp, scale=1.0
            )

            # ---- projections: kt/qt = [xjT | xmT] per grp, bf16 --------
            # weight block order: k before q so mm1 deps resolve earliest
            kt, qt = [None, None], [None, None]
            copy_engines = [nc.vector, nc.scalar, nc.gpsimd, nc.vector]
            pt_list = []
            for i, (nm, grp) in enumerate([("k", 0), ("q", 0), ("k", 1), ("q", 1)]):
                blk = WBLK + (0 if nm == "k" else 4 * 128) + grp * 2 * 128
                ps = ps_proj.tile([D, 120], F32, tag="proj")
                nc.tensor.matmul(
                    out=ps[:, 0:J], lhsT=inp_sb[:, blk : blk + D], rhs=ejT
                )
                nc.tensor.matmul(
                    out=ps[:, J : J + M],
                    lhsT=inp_sb[:, blk + D : blk + 2 * D],
                    rhs=emT,
                )
                sb = pp.tile([D, 120], BF16, tag=f"{nm}t{grp}")
                pt_list.append((copy_engines[i], sb, ps))
                (kt if nm == "k" else qt)[grp] = sb

            # uv vectors: uv_ps rows 0:100 <- ej @ uwj, rows 100:120 <- em @ uwm
            uv_ps = ps_out.tile([120, 16], F32, tag="uv")
            nc.tensor.matmul(
                out=uv_ps[0:120, 0:8],
                lhsT=inp_sb[:, 0:120],
                rhs=inp_sb[:, UWJ : UWJ + 8],
            )
            nc.tensor.matmul(
                out=uv_ps[0:120, 8:16],
                lhsT=inp_sb[:, 0:120],
                rhs=inp_sb[:, UWM : UWM + 8],
            )
            for eng, sb, ps in pt_list:
                if eng is nc.gpsimd:
                    eng.tensor_copy(out=sb, in_=ps)
                elif eng is nc.scalar:
                    eng.copy(out=sb, in_=ps)
                else:
                    eng.tensor_copy(out=sb, in_=ps)
            uv_sb = pp.tile([120, 16], F32, tag="uvsb")
            nc.vector.tensor_copy(out=uv_sb, in_=uv_ps)

            # ---- attention: 2 groups of 4 heads ------------------------
            s_ps = ps_out.tile([J, H, 40], F32, tag="sE")
            f_ps = ps_out.tile([J, H, 40], F32, tag="sF")
            for G in range(2):
                psG = ps_att.tile([120, 480], F32, tag="att")
                for g in range(4):
                    nc.tensor.matmul(
                        out=psG[0:120, 120 * g : 120 * g + 120],
                        lhsT=kt[G][32 * g : 32 * g + 32, 0:120],
                        rhs=qt[G][32 * g : 32 * g + 32, 0:120],
                        tile_position=(32 * g, 0),
                    )
                e1 = rp.tile([120, 560], BF16, tag="e1")
                nc.scalar.activation(
                    out=e1[0:120, 0:480],
                    in_=psG[0:120, 0:480],
                    func=AF.Exp,
                    scale=INV_SQ,
                )
                for g in range(4):
                    h = 4 * G + g
                    c0 = 120 * g
                    # E-side scaled copy (in place, chunk2-addressable)
                    nc.vector.tensor_scalar_mul(
                        out=e1[0:J, 480 + 20 * g : 500 + 20 * g],
                        in0=e1[0:J, c0 + 100 : c0 + 120],
                        scalar1=uv_sb[0:J, h : h + 1],
                    )
                    # F-side rhs: [eDT | eDT*uvm] in zero-framed rows 100:120
                    nc.gpsimd.tensor_copy(
                        out=rz[h][100:120, 0:20], in_=e1[100:120, c0 + 100 : c0 + 120]
                    )
                    nc.vector.tensor_scalar_mul(
                        out=rz[h][100:120, 20:40],
                        in0=e1[100:120, c0 + 100 : c0 + 120],
                        scalar1=uv_sb[100:120, 8 + h : 9 + h],
                    )
                    # E: [SE|Nj] = eAT.T @ [eCT | eCT*uvj]   (K=100)
                    nc.tensor.matmul(
                        out=s_ps[0:J, h, :],
                        lhsT=e1[0:J, c0 : c0 + J],
                        rhs=_chunk2(e1[0:J, c0 + 100 : c0 + 120], 380 - 100 * g),
                    )
                    # F: [SF|Nm] = [eAT;eBT].T @ zero-framed [eDT | eDT*uvm]
                    nc.tensor.matmul(
                        out=f_ps[0:J, h, :],
                        lhsT=e1[0:120, c0 : c0 + J],
                        rhs=rz[h][0:120, 0:40],
                    )

            def pmh(ap3):  # [p, h, m] -> [p, m, h]
                return ap3.rearrange("p h m -> p m h")

            # ---- combine: c1 = sum_h Nj/SE + Nm/SF ---------------------
            ratE = pp.tile([J, M, H], F32, tag="ratE")
            ratF = pp.tile([J, M, H], F32, tag="ratF")
            if USE_DIVIDE:
                nc.vector.tensor_tensor(
                    out=ratE,
                    in0=pmh(s_ps[:, :, 20:40]),
                    in1=pmh(s_ps[:, :, 0:20]),
                    op=OP.divide,
                )
                nc.gpsimd.tensor_tensor(
                    out=ratF,
                    in0=pmh(f_ps[:, :, 20:40]),
                    in1=pmh(f_ps[:, :, 0:20]),
                    op=OP.divide,
                )
            else:
                rE = pp.tile([J, M, H], F32, tag="rE")
                nc.vector.reciprocal(out=rE, in_=pmh(s_ps[:, :, 0:20]))
                nc.vector.tensor_mul(out=ratE, in0=pmh(s_ps[:, :, 20:40]), in1=rE)
                rF = pp.tile([J, M, H], F32, tag="rF")
                nc.gpsimd.reciprocal(out=rF, in_=pmh(f_ps[:, :, 0:20]))
                nc.gpsimd.tensor_mul(out=ratF, in0=pmh(f_ps[:, :, 20:40]), in1=rF)
            c8 = pp.tile([J, M, H], F32, tag="c8")
            nc.vector.tensor_add(out=c8, in0=ratE, in1=ratF)
            c1 = pp.tile([J, M], F32, tag="c1")
            nc.vector.reduce_sum(out=c1, in_=c8, axis=AX.X)

            # ---- logits = 10*tanh((c1+bias)/sqrt(D)) + mask; softmax ---
            th = pp.tile([J, M], F32, tag="th")
            nc.scalar.activation(
                out=th,
                in_=c1,
                func=AF.Tanh,
                scale=1.0 / SD,
                bias=inp_sb[0:J, BC : BC + 1],
            )
            e10 = pp.tile([J, M], F32, tag="e10")
            nc.scalar.activation(out=e10, in_=th, func=AF.Exp, scale=10.0)
            e_sb = pp.tile([J, M], F32, tag="esb")
            s_row = pp.tile([J, 1], F32, tag="srow")
            nc.vector.scalar_tensor_tensor(
                out=e_sb,
                in0=e10,
                scalar=1.0,
                in1=expmask,
                op0=OP.mult,
                op1=OP.mult,
                accum_out=s_row,
            )
            tot_ps = ps_out.tile([J, 1], F32, tag="tot")
            nc.tensor.matmul(out=tot_ps[0:J, 0:1], lhsT=ones, rhs=s_row)
            out_t = pp.tile([J, M], F32, tag="outt")
            if USE_DIVIDE:
                nc.vector.tensor_scalar(
                    out=out_t,
                    in0=e_sb,
                    scalar1=tot_ps[0:J, 0:1],
                    scalar2=None,
                    op0=OP.divide,
                )
            else:
                rtot = pp.tile([J, 1], F32, tag="rtot")
                nc.vector.reciprocal(out=rtot, in_=tot_ps[0:J, 0:1])
                nc.vector.tensor_scalar_mul(out=out_t, in0=e_sb, scalar1=rtot)
            nc.sync.dma_start(out=out_d[:], in_=out_t)

    _split_multi_waits(nc)
    return nc


def _pack_wblk(w):
    """[128, 64] head-major weight half -> padded 32-strip [128, 128] block."""
    blk = np.zeros((D, D), np.float32)
    for g in range(4):
        blk[:, 32 * g : 32 * g + 16] = w[:, 16 * g : 16 * g + 16]
    return blk


_NC = None
last_results = None


def kernel(**inputs):
    global _NC, last_results
    _install_drain_patch()
    if _NC is None:
        _NC = _build()

    f32 = np.float32
    Wq3 = np.asarray(inputs["Wq3"], f32)
    Wk = np.asarray(inputs["Wk"], f32)
    Wv = np.asarray(inputs["Wv"], f32)
    Wmhc = np.asarray(inputs["Wmhc"], f32)
    b_mhc = np.asarray(inputs["b_mhc"], f32).reshape(D)
    Wshc = np.asarray(inputs["Wshc"], f32).reshape(D)
    b_shc = float(np.asarray(inputs["b_shc"]).reshape(-1)[0])

    w2 = Wmhc @ Wshc  # [128]
    bias_c = float(b_mhc @ Wshc + b_shc)
    uwj = np.stack(
        [Wv[:D, 16 * h : 16 * h + 16] @ w2[16 * h : 16 * h + 16] for h in range(H)], 1
    )
    uwm = np.stack(
        [Wv[D:, 16 * h : 16 * h + 16] @ w2[16 * h : 16 * h + 16] for h in range(H)], 1
    )

    base = np.zeros((D, NCOL), f32)
    off = WBLK
    for wj, wm in ((Wk[:D], Wk[D:]), (Wq3[:D], Wq3[D:])):
        for grp in range(2):
            for w in (wj, wm):
                base[:, off : off + D] = _pack_wblk(w[:, 64 * grp : 64 * grp + 64])
                off += D
    base[:, UWJ : UWJ + 8] = uwj
    base[:, UWM : UWM + 8] = uwm
    base[0:J, BC] = bias_c / SD

    ejs = np.asarray(inputs["encoded_job"], f32)
    ems = np.asarray(inputs["encoded_machine"], f32)
    msks = np.asarray(inputs["ninf_mask"], f32)

    in_maps = []
    for b in range(B):
        ed = base.copy()
        ed[:, EJ : EJ + J] = ejs[b].T
        ed[:, EM : EM + M] = ems[b].T
        ed[0:J, MK : MK + M] = msks[b]
        in_maps.append({"inp": ed.astype(ml_dtypes.bfloat16)})

    last_results = run_bass_kernel_spmd(_NC, in_maps, core_ids=list(range(B)))
    out = np.stack(
        [last_results.results[b]["out"].reshape(J * M) for b in range(B)]
    )
    return out.astype(np.float32)


# revision 39
# speedup vs baseline: 1.6410x; 1.0602x over previous
"""FJSP decoder kernel for Trainium2, data-parallel over batch on 8 NeuronCores.

Factorized attention (see derivation in comments): q/k/v of the flattened
(job, machine) pair s=(j,m) split as x[s] = xj[j] + xm[m], so the joint
softmax over t=(j',m') factorizes exactly:

  exp(score[s,t]) = expE[s,j'] * expF[s,m']
  softmax_t(score) @ v . w2 = Nj/SE + Nm/SF      (per head)

with expE[(j,m),j'] = eA[j,j']*eC[m,j'], expF[(j,m),m'] = eB[j,m']*eD[m,m'].
The multi-head combine collapses through w2 = Wmhc @ Wshc, so v only enters
via uv = x @ (Wv_blocks @ w2) -- the v projection never runs on device.

Device-side layout: per head h (grp=h//4, strip g=h%4) one joint matmul with
stationary [kjT | kmT] ([32, 120]) against rhs [qjT | qmT] gives the full
[120, 120] block (rows 0:100 = A^T,C^T; rows 100:120 = B^T,D^T); one exp per
4-head group covers everything.  E-side contraction (K=100) reads the exp
tile in place via a 2-chunk AP; the F-side (K=120) uses a zero-framed rhs so
the B^T rows land in the same matmul.  All matmul operands are bf16 (4x PE
throughput vs f32); final softmax chain stays f32.

Host-side prep is layout/weight-folding only: weights pre-padded into the
32-strip head layout, activations pre-transposed, w2/uw folded.  One input
DMA, one output DMA.
"""

import math

import numpy as np
import ml_dtypes

import concourse.bass as bass
import concourse.mybir as mybir
import concourse.tile as tile
from concourse.bass_utils import run_bass_kernel_spmd

F32 = mybir.dt.float32
BF16 = mybir.dt.bfloat16
AF = mybir.ActivationFunctionType
OP = mybir.AluOpType
AX = mybir.AxisListType

D, H, QD = 128, 8, 16
B, J, M = 8, 100, 20
INV_SQ = 1.0 / math.sqrt(QD)  # 0.25
SD = math.sqrt(D)

# input column layout (all bf16, [128, NCOL])
EJ = 0                      # ejT [0:128, 0:100]
EM = 100                    # emT [0:128, 100:120]
WBLK = 120                  # 8 weight blocks of 128 cols each:
#   order: (k,j,g0) (k,m,g0) (k,j,g1) (k,m,g1) (q,j,g0) (q,m,g0) (q,j,g1) (q,m,g1)
UWJ = WBLK + 8 * 128        # 1144: uwj [0:128, 8]
UWM = UWJ + 8               # 1152: uwm [0:128, 8]
MK = UWM + 8                # 1160: mask [0:100, 20]
BC = MK + 20                # 1180: bias col [0:100, 1] = bias_c / sqrt(D)
NCOL = BC + 1               # 1181

# walrus ISA check rejects divide ALU ops on DVE; keep reciprocal+mul
USE_DIVIDE = False


# ---------------------------------------------------------------------------
# gen3 walrus accepts one sync-wait per instruction. Tile's kernel-tail
# drain accumulates one wait per active logical processor on a single
# Drain: spread them across engines (parallel waiting). Tile's semaphore
# pass can also attach >1 wait to ordinary instructions: shed extras onto
# same-engine NoOps inserted right before the offender.
_PATCHED = False


def _install_drain_patch():
    global _PATCHED
    if _PATCHED:
        return
    from concourse.tile import ScopedClock, TileContext

    def _split_drain_and_barrier(self, tick_clock, wait_clock):
        drain_inst = self.nc.sync.drain()
        wait_clock.add_sem_waits(
            drain_inst.ins, ScopedClock({None: tick_clock.global_clock})
        )
        si = drain_inst.ins.sync_info
        waits = list(si.on_wait) if si is not None else []
        if len(waits) > 1:
            assert not si.on_update
            sems = {s.name: s for s in self.sems.allocated().values()}
            drain_inst.ins.sync_info = None
            drain_inst.wait_op(sems[waits[0].ant_name], waits[0].wait_value, "sem-ge")
            engines = [
                self.nc.scalar,
                self.nc.vector,
                self.nc.tensor,
                self.nc.gpsimd,
                self.nc.sync,
            ]
            for i, w in enumerate(waits[1:]):
                extra = engines[i % len(engines)].drain()
                extra.wait_op(sems[w.ant_name], w.wait_value, "sem-ge")
        self.nc.all_engine_barrier()
        assert self.sems is not None
        popped = self.nc._tile_sem_poison_stack.pop()
        assert popped is self._sem_poison
        self.nc.clear_and_free_semaphores(list(self.sems.allocated().values()))

    TileContext._drain_and_barrier = _split_drain_and_barrier
    _PATCHED = True


def _split_multi_waits(nc):
    import bass_rust

    ctr = 0
    for fn in nc.m.functions:
        for bb in fn.blocks:
            il = bb.instructions
            if not any(
                i.sync_info is not None and len(i.sync_info.on_wait) > 1 for i in il
            ):
                continue
            new = []
            for ins in il:
                si = ins.sync_info
                if si is not None and len(si.on_wait) > 1:
                    waits = list(si.on_wait)
                    ups = list(si.on_update)
                    for w in waits[:-1]:
                        nop = mybir.InstNoOp(name=f"I-waitsplit-{ctr}", ins=[], outs=[])
                        ctr += 1
                        nop.engine = ins.engine
                        nop.sync_info = bass_rust.SyncInfo(on_update=[], on_wait=[w])
                        new.append(nop)
                    ins.sync_info = bass_rust.SyncInfo(
                        on_update=ups, on_wait=[waits[-1]]
                    )
                new.append(ins)
            bb.instructions = new


def _hoist_input_dma(nc):
    """Move the input DMACopy from the body block into the preamble block,
    right after the sequencer register-init moves and before the entry
    barrier. The DMA has no waits and its completion semaphore gates all
    consumers, so issuing it ~800ns earlier (in parallel with the barrier)
    is safe and shortens the critical path by the same amount."""
    fn = nc.m.functions[0]
    if len(fn.blocks) < 2:
        return
    b0, b1 = fn.blocks[0], fn.blocks[1]
    dma = None
    for ins in b1.instructions:
        if type(ins).__name__ == "InstDMACopy":
            si = ins.sync_info
            assert si is None or not si.on_wait
            dma = ins
            break
    if dma is None:
        return
    b1.instructions = [i for i in b1.instructions if i is not dma]
    pos = 1 if b0.instructions and type(b0.instructions[0]).__name__ == "InstCall" else 0
    b0.instructions = b0.instructions[:pos] + [dma] + b0.instructions[pos:]


def _ap_free_range(ap_obj):
    """[lo, hi) element range of an AP's free dims (dim 0 = partitions)."""
    lo = ap_obj.offset
    hi = lo + 1
    for stride, count in list(ap_obj.ap)[1:]:
        hi += stride * (count - 1)
    return lo, hi


def _tighten_psum_waits(nc):
    """The tile scheduler bakes each instruction's PE-tick wait from its
    scheduled slot, which over-approximates for combine ops: they end up
    waiting on unrelated later matmuls into the same (or another) PSUM
    tile. Recompute the true minimal PE tick for DVE readers of the
    mm3/mm4 tiles (cA/cB) from AP range overlap with the PE writers."""
    fn = nc.m.functions[0]
    pe_sem = None
    cnt = 0
    writers = {}  # memref -> [(lo, hi, tick)]
    for bb in fn.blocks:
        for ins in bb.instructions:
            si = ins.sync_info
            if str(ins.engine) != "EngineType.PE" or si is None:
                continue
            for u in si.on_update:
                if pe_sem is None and u.ant_name.startswith("PE"):
                    pe_sem = u.ant_name
                if u.ant_name == pe_sem:
                    cnt += u.update_value
            outs = getattr(ins, "outs", [])
            if outs:
                mr = str(getattr(outs[0], "memref", ""))
                if mr.startswith(("cA", "cB")):
                    lo, hi = _ap_free_range(outs[0])
                    writers.setdefault(mr, []).append((lo, hi, cnt))
    if pe_sem is None or not writers:
        return
    for bb in fn.blocks:
        for ins in bb.instructions:
            si = ins.sync_info
            if str(ins.engine) != "EngineType.DVE" or si is None:
                continue
            srcs = getattr(ins, "ins", [])
            if not srcs:
                continue
            mr = str(getattr(srcs[0], "memref", ""))
            if mr not in writers:
                continue
            lo, hi = _ap_free_range(srcs[0])
            need = 0
            for wlo, whi, tick in writers[mr]:
                if wlo < hi and lo < whi:
                    need = max(need, tick)
            for w in si.on_wait:
                if w.ant_name == pe_sem and w.wait_value > need > 0:
                    w.wait_value = need


def _chunk2(ap_slice, chunk_step):
    """Matmul rhs built from two equal column chunks `chunk_step` apart."""
    return bass.AP(
        tensor=ap_slice.tensor,
        offset=ap_slice.offset,
        ap=[ap_slice.ap[0], [chunk_step, 2], ap_slice.ap[1]],
    )


def _build():
    nc = bass.Bass()
    inp_d = nc.dram_tensor("inp", [D, NCOL], BF16, kind="ExternalInput")
    out_d = nc.dram_tensor("out", [J, M], F32, kind="ExternalOutput")

    with tile.TileContext(nc) as tc:
        with (
            tc.tile_pool(name="persist", bufs=1) as pp,
            tc.tile_pool(name="eero", bufs=4) as rp,
            tc.tile_pool(name="ps_proj", bufs=2, space="PSUM") as ps_proj,
            tc.tile_pool(name="ps_att", bufs=3, space="PSUM") as ps_att,
            tc.tile_pool(name="ps_out", bufs=1, space="PSUM") as ps_out,
        ):
            # ---- single input DMA, issued first ------------------------
            inp_sb = pp.tile([D, NCOL], BF16, tag="inp")
            nc.sync.dma_start(out=inp_sb, in_=inp_d[:])

            # ---- constants (no input dependency; overlap the DMA) ------
            ones = pp.tile([J, J], F32, tag="ones")
            nc.gpsimd.memset(ones, 1.0)
            # zmask: 1 on the valid m' rows 100:120, 0 on the 96:100 slack
            # (engine partition bases must be 32-aligned, so all ops on the
            # m'-rows touch the superset [96:120] and mask out 96:100)
            zmask = pp.tile([120, 1], F32, tag="zmask")
            nc.gpsimd.memset(zmask, 1.0)
            nc.gpsimd.memset(zmask[96:100, :], 0.0)
            rz = []
            for h in range(H):
                t = pp.tile([120, 40], BF16, tag=f"rz{h}")
                nc.gpsimd.memset(t[0:100, :], 0.0)
                rz.append(t)

            ejT = inp_sb[:, EJ : EJ + J]
            emT = inp_sb[:, EM : EM + M]

            # exp(mask): off the critical path, folds the mask add into the
            # final softmax as a multiply
            expmask = pp.tile([J, M], F32, tag="expmask")
            nc.scalar.activation(
                out=expmask, in_=inp_sb[0:J, MK : MK + M], func=AF.Exp, scale=1.0
            )

            # ---- projections: kq[G] = [kjT|kmT | qjT|qmT] per grp, bf16 -
            # one shared PSUM tile + one copy per grp (copies charge by
            # columns, so packing k and q halves the copy instructions)
            kt, qt = [None, None], [None, None]
            pt_list = []
            for grp in range(2):
                ps = ps_proj.tile([D, 240], F32, tag="proj")
                for half, nm in enumerate(("k", "q")):
                    blk = WBLK + (0 if nm == "k" else 4 * 128) + grp * 2 * 128
                    nc.tensor.matmul(
                        out=ps[:, 120 * half : 120 * half + J],
                        lhsT=inp_sb[:, blk : blk + D],
                        rhs=ejT,
                    )
                    nc.tensor.matmul(
                        out=ps[:, 120 * half + J : 120 * half + J + M],
                        lhsT=inp_sb[:, blk + D : blk + 2 * D],
                        rhs=emT,
                    )
                sb = pp.tile([D, 240], BF16, tag=f"kq{grp}")
                pt_list.append((nc.vector if grp == 0 else nc.scalar, sb, ps))
                kt[grp] = sb[:, 0:120]
                qt[grp] = sb[:, 120:240]

            # uv vectors: uv_ps rows 0:100 <- ej @ uwj, rows 100:120 <- em @ uwm
            uv_ps = ps_out.tile([120, 17], F32, tag="uv")
            nc.tensor.matmul(
                out=uv_ps[0:120, 0:8],
                lhsT=inp_sb[:, 0:120],
                rhs=inp_sb[:, UWJ : UWJ + 8],
            )
            nc.tensor.matmul(
                out=uv_ps[0:120, 8:16],
                lhsT=inp_sb[:, 0:120],
                rhs=inp_sb[:, UWM : UWM + 8],
            )
            for eng, sb, ps in pt_list:
                if eng is nc.scalar:
                    eng.copy(out=sb[:, 0:120], in_=ps[:, 0:120])
                    nc.vector.tensor_copy(out=sb[:, 120:240], in_=ps[:, 120:240])
                else:
                    eng.tensor_copy(out=sb, in_=ps)
            uv_sb = pp.tile([120, 16], F32, tag="uvsb")
            nc.vector.tensor_copy(out=uv_sb, in_=uv_ps[0:120, 0:16])
            # zero the m'-side uv rows in the 96:100 slack so masked TSPs
            # reading [96:120] produce exact zeros there
            nc.vector.memset(uv_sb[96:100, 8:16], 0.0)

            # ---- attention: 4 strip-pairs (head g with head 4+g) -------
            # HW constraint: a PSUM tile must not mix different tile_position
            # values, and heads g / 4+g share tile_position (32g, 0) -- so
            # each strip-pair gets its own mm1 PSUM tile and one fused exp.
            # Per strip: ps_c[g][0:J, G, 0:80] = [SE|Nj|SF|Nm]; combine for
            # strip g runs right after its mm3/mm4 so strips 0-2 hide under
            # later strips' attention.
            ps_c = []
            for pname in ("cA", "cB"):
                ps_g = ps_out.tile([J, 4, 80], F32, tag=pname, name=pname)
                ps_c.append(ps_g)
            # ratAll[j, strip, G, E/F, m]; one fused XYZ-reduce at the end
            ratAll = pp.tile([J, 4, 2, 2, M], F32, tag="ratAll")

            for g in range(4):
                psP = ps_att.tile([120, 240], F32, tag="att")
                e1 = rp.tile([120, 280], BF16, tag="e1")
                for G in range(2):
                    nc.tensor.matmul(
                        out=psP[0:120, 120 * G : 120 * G + 120],
                        lhsT=kt[G][32 * g : 32 * g + 32, 0:120],
                        rhs=qt[G][32 * g : 32 * g + 32, 0:120],
                        tile_position=(32 * g, 0),
                    )
                nc.scalar.activation(
                    out=e1[0:120, 0:240],
                    in_=psP[0:120, 0:240],
                    func=AF.Exp,
                    scale=INV_SQ,
                )
                # F-rhs construction engine balance: Pool serializes ~123ns
                # per op, so late strips spread across DVE (66ns) and Act
                # (Copy-with-scale, fills its post-exp idle)
                for G in range(2):
                    h = 4 * G + g
                    c0 = 120 * G
                    # E-side scaled copy (in place, chunk2-addressable)
                    nc.vector.tensor_scalar_mul(
                        out=e1[0:J, 240 + 20 * G : 260 + 20 * G],
                        in0=e1[0:J, c0 + 100 : c0 + 120],
                        scalar1=uv_sb[0:J, h : h + 1],
                    )
                    # F-side rhs [eDT | eDT*uvm] into zero-framed rows
                    # 100:120 via the 96-aligned superset; zmask / zeroed uv
                    # rows keep the 96:100 slack exactly zero.
                    if g == 3 and G == 0:
                        nc.scalar.activation(
                            out=rz[h][96:120, 0:20],
                            in_=e1[96:120, c0 + 100 : c0 + 120],
                            func=AF.Copy, scale=zmask[96:120, 0:1],
                        )
                        nc.scalar.activation(
                            out=rz[h][96:120, 20:40],
                            in_=e1[96:120, c0 + 100 : c0 + 120],
                            func=AF.Copy, scale=uv_sb[96:120, 8 + h : 9 + h],
                        )
                    else:
                        feng = nc.vector if G == 1 else nc.gpsimd
                        feng.tensor_scalar_mul(
                            out=rz[h][96:120, 0:20],
                            in0=e1[96:120, c0 + 100 : c0 + 120],
                            scalar1=zmask[96:120, 0:1],
                        )
                        feng.tensor_scalar_mul(
                            out=rz[h][96:120, 20:40],
                            in0=e1[96:120, c0 + 100 : c0 + 120],
                            scalar1=uv_sb[96:120, 8 + h : 9 + h],
                        )
                    # E: [SE|Nj] = eAT.T @ [eCT | eCT*uvj]   (K=100)
                    nc.tensor.matmul(
                        out=ps_c[g // 2][0:J, 2 * (g % 2) + G, 0:40],
                        lhsT=e1[0:J, c0 : c0 + J],
                        rhs=_chunk2(e1[0:J, c0 + 100 : c0 + 120], 140 - 100 * G),
                    )
                    # F: [SF|Nm] = [eAT;eBT].T @ zero-framed [eDT | eDT*uvm]
                    nc.tensor.matmul(
                        out=ps_c[g // 2][0:J, 2 * (g % 2) + G, 40:80],
                        lhsT=e1[0:120, c0 : c0 + J],
                        rhs=rz[h][0:120, 0:40],
                    )
            # ---- combine: pair A fused; pair B per strip so only strip
            # 3's recip+mul sit after the last mm4. Wait-floors steer the
            # greedy FIFO scheduler: strip-3's TSPs must come first on DVE.
            t5A = ps_c[0][0:J, :, :].rearrange("p h (a b m) -> p h a b m", a=2, b=2)
            rA = pp.tile([J, 4, 2, M], F32, tag="rA")
            with tc.tile_wait_until(0.0059):
                nc.vector.reciprocal(out=rA, in_=t5A[:, :, :, 0, :])
            with tc.tile_wait_until(0.0064):
                nc.vector.scalar_tensor_tensor(
                    out=ratAll[0:J, 0:2, :, :, :], in0=t5A[:, :, :, 1, :],
                    scalar=1.0, in1=rA, op0=OP.mult, op1=OP.mult,
                )
            t5B = ps_c[1][0:J, :, :].rearrange("p h (a b m) -> p h a b m", a=2, b=2)
            rB = pp.tile([J, 4, 2, M], F32, tag="rB")
            nc.vector.reciprocal(out=rB, in_=t5B[:, :, :, 0, :])
            nc.vector.scalar_tensor_tensor(
                out=ratAll[0:J, 2:4, :, :, :], in0=t5B[:, :, :, 1, :],
                scalar=1.0, in1=rB, op0=OP.mult, op1=OP.mult,
            )

            # ---- combine tail: c1 = sum over (strip, G, E/F) -----------
            c1 = pp.tile([J, M], F32, tag="c1")
            nc.vector.reduce_sum(
                out=c1, in_=ratAll.rearrange("p s g a m -> p m s g a"),
                axis=AX.XYZ,
            )

            # ---- logits = 10*tanh((c1+bias)/sqrt(D)) + mask; softmax ---
            th = pp.tile([J, M], F32, tag="th")
            nc.scalar.activation(
                out=th,
                in_=c1,
                func=AF.Tanh,
                scale=1.0 / SD,
                bias=inp_sb[0:J, BC : BC + 1],
            )
            e10 = pp.tile([J, M], F32, tag="e10")
            nc.scalar.activation(out=e10, in_=th, func=AF.Exp, scale=10.0)
            e_sb = pp.tile([J, M], F32, tag="esb")
            s_row = pp.tile([J, 1], F32, tag="srow")
            nc.vector.scalar_tensor_tensor(
                out=e_sb,
                in0=e10,
                scalar=1.0,
                in1=expmask,
                op0=OP.mult,
                op1=OP.mult,
                accum_out=s_row,
            )
            tot_ps = uv_ps[0:J, 16:17]
            nc.tensor.matmul(out=tot_ps, lhsT=ones, rhs=s_row)
            out_t = pp.tile([J, M], F32, tag="outt")
            if USE_DIVIDE:
                nc.vector.tensor_scalar(
                    out=out_t,
                    in0=e_sb,
                    scalar1=tot_ps,
                    scalar2=None,
                    op0=OP.divide,
                )
            else:
                rtot = pp.tile([J, 1], F32, tag="rtot")
                nc.vector.reciprocal(out=rtot, in_=tot_ps)
                nc.vector.tensor_scalar_mul(out=out_t, in0=e_sb, scalar1=rtot)
            nc.sync.dma_start(out=out_d[:], in_=out_t)

    _split_multi_waits(nc)
    _hoist_input_dma(nc)
    _tighten_psum_waits(nc)
    return nc


def _pack_wblk(w):
    """[128, 64] head-major weight half -> padded 32-strip [128, 128] block."""
    blk = np.zeros((D, D), np.float32)
    for g in range(4):
        blk[:, 32 * g : 32 * g + 16] = w[:, 16 * g : 16 * g + 16]
    return blk


_NC = None
last_results = None


def kernel(**inputs):
    global _NC, last_results
    _install_drain_patch()
    if _NC is None:
        _NC = _build()

    f32 = np.float32
    Wq3 = np.asarray(inputs["Wq3"], f32)
    Wk = np.asarray(inputs["Wk"], f32)
    Wv = np.asarray(inputs["Wv"], f32)
    Wmhc = np.asarray(inputs["Wmhc"], f32)
    b_mhc = np.asarray(inputs["b_mhc"], f32).reshape(D)
    Wshc = np.asarray(inputs["Wshc"], f32).reshape(D)
    b_shc = float(np.asarray(inputs["b_shc"]).reshape(-1)[0])

    w2 = Wmhc @ Wshc  # [128]
    bias_c = float(b_mhc @ Wshc + b_shc)
    uwj = np.stack(
        [Wv[:D, 16 * h : 16 * h + 16] @ w2[16 * h : 16 * h + 16] for h in range(H)], 1
    )
    uwm = np.stack(
        [Wv[D:, 16 * h : 16 * h + 16] @ w2[16 * h : 16 * h + 16] for h in range(H)], 1
    )

    base = np.zeros((D, NCOL), f32)
    off = WBLK
    for wj, wm in ((Wk[:D], Wk[D:]), (Wq3[:D], Wq3[D:])):
        for grp in range(2):
            for w in (wj, wm):
                base[:, off : off + D] = _pack_wblk(w[:, 64 * grp : 64 * grp + 64])
                off += D
    base[:, UWJ : UWJ + 8] = uwj
    base[:, UWM : UWM + 8] = uwm
    base[0:J, BC] = bias_c / SD

    ejs = np.asarray(inputs["encoded_job"], f32)
    ems = np.asarray(inputs["encoded_machine"], f32)
    msks = np.asarray(inputs["ninf_mask"], f32)

    in_maps = []
    for b in range(B):
        ed = base.copy()
        ed[:, EJ : EJ + J] = ejs[b].T
        ed[:, EM : EM + M] = ems[b].T
        ed[0:J, MK : MK + M] = msks[b]
        in_maps.append({"inp": ed.astype(ml_dtypes.bfloat16)})

    last_results = run_bass_kernel_spmd(_NC, in_maps, core_ids=list(range(B)))
    out = np.stack(
        [last_results.results[b]["out"].reshape(J * M) for b in range(B)]
    )
    return out.astype(np.float32)


# revision 40
# speedup vs baseline: 1.7013x; 1.0368x over previous
"""FJSP decoder kernel for Trainium2, data-parallel over batch on 8 NeuronCores.

Factorized attention (see derivation in comments): q/k/v of the flattened
(job, machine) pair s=(j,m) split as x[s] = xj[j] + xm[m], so the joint
softmax over t=(j',m') factorizes exactly:

  exp(score[s,t]) = expE[s,j'] * expF[s,m']
  softmax_t(score) @ v . w2 = Nj/SE + Nm/SF      (per head)

with expE[(j,m),j'] = eA[j,j']*eC[m,j'], expF[(j,m),m'] = eB[j,m']*eD[m,m'].
The multi-head combine collapses through w2 = Wmhc @ Wshc, so v only enters
via uv = x @ (Wv_blocks @ w2) -- the v projection never runs on device.

Device-side layout: per head h (grp=h//4, strip g=h%4) one joint matmul with
stationary [kjT | kmT] ([32, 120]) against rhs [qjT | qmT] gives the full
[120, 120] block (rows 0:100 = A^T,C^T; rows 100:120 = B^T,D^T); one exp per
4-head group covers everything.  E-side contraction (K=100) reads the exp
tile in place via a 2-chunk AP; the F-side (K=120) uses a zero-framed rhs so
the B^T rows land in the same matmul.  All matmul operands are bf16 (4x PE
throughput vs f32); final softmax chain stays f32.

Host-side prep is layout/weight-folding only: weights pre-padded into the
32-strip head layout, activations pre-transposed, w2/uw folded.  One input
DMA, one output DMA.
"""

import math

import numpy as np
import ml_dtypes

import concourse.bass as bass
import concourse.mybir as mybir
import concourse.tile as tile
from concourse.bass_utils import run_bass_kernel_spmd

F32 = mybir.dt.float32
BF16 = mybir.dt.bfloat16
AF = mybir.ActivationFunctionType
OP = mybir.AluOpType
AX = mybir.AxisListType

D, H, QD = 128, 8, 16
B, J, M = 8, 100, 20
INV_SQ = 1.0 / math.sqrt(QD)  # 0.25
SD = math.sqrt(D)

# input column layout (all bf16, [128, NCOL])
EJ = 0                      # ejT [0:128, 0:100]
EM = 100                    # emT [0:128, 100:120]
WBLK = 120                  # 8 weight blocks of 128 cols each:
#   order: (k,j,g0) (k,m,g0) (k,j,g1) (k,m,g1) (q,j,g0) (q,m,g0) (q,j,g1) (q,m,g1)
UWJ = WBLK + 8 * 128        # 1144: uwj [0:128, 8]
UWM = UWJ + 8               # 1152: uwm [0:128, 8]
MK = UWM + 8                # 1160: mask [0:100, 20]
BC = MK + 20                # 1180: bias col [0:100, 1] = bias_c / sqrt(D)
NCOL = BC + 1               # 1181

# walrus ISA check rejects divide ALU ops on DVE; keep reciprocal+mul
USE_DIVIDE = False


# ---------------------------------------------------------------------------
# gen3 walrus accepts one sync-wait per instruction. Tile's kernel-tail
# drain accumulates one wait per active logical processor on a single
# Drain: spread them across engines (parallel waiting). Tile's semaphore
# pass can also attach >1 wait to ordinary instructions: shed extras onto
# same-engine NoOps inserted right before the offender.
_PATCHED = False


def _install_drain_patch():
    global _PATCHED
    if _PATCHED:
        return
    from concourse.tile import ScopedClock, TileContext

    def _split_drain_and_barrier(self, tick_clock, wait_clock):
        drain_inst = self.nc.sync.drain()
        wait_clock.add_sem_waits(
            drain_inst.ins, ScopedClock({None: tick_clock.global_clock})
        )
        si = drain_inst.ins.sync_info
        waits = list(si.on_wait) if si is not None else []
        if len(waits) > 1:
            assert not si.on_update
            sems = {s.name: s for s in self.sems.allocated().values()}
            drain_inst.ins.sync_info = None
            drain_inst.wait_op(sems[waits[0].ant_name], waits[0].wait_value, "sem-ge")
            engines = [
                self.nc.scalar,
                self.nc.vector,
                self.nc.tensor,
                self.nc.gpsimd,
                self.nc.sync,
            ]
            for i, w in enumerate(waits[1:]):
                extra = engines[i % len(engines)].drain()
                extra.wait_op(sems[w.ant_name], w.wait_value, "sem-ge")
        self.nc.all_engine_barrier()
        assert self.sems is not None
        popped = self.nc._tile_sem_poison_stack.pop()
        assert popped is self._sem_poison
        self.nc.clear_and_free_semaphores(list(self.sems.allocated().values()))

    TileContext._drain_and_barrier = _split_drain_and_barrier
    _PATCHED = True


def _split_multi_waits(nc):
    import bass_rust

    ctr = 0
    for fn in nc.m.functions:
        for bb in fn.blocks:
            il = bb.instructions
            if not any(
                i.sync_info is not None and len(i.sync_info.on_wait) > 1 for i in il
            ):
                continue
            new = []
            for ins in il:
                si = ins.sync_info
                if si is not None and len(si.on_wait) > 1:
                    waits = list(si.on_wait)
                    ups = list(si.on_update)
                    for w in waits[:-1]:
                        nop = mybir.InstNoOp(name=f"I-waitsplit-{ctr}", ins=[], outs=[])
                        ctr += 1
                        nop.engine = ins.engine
                        nop.sync_info = bass_rust.SyncInfo(on_update=[], on_wait=[w])
                        new.append(nop)
                    ins.sync_info = bass_rust.SyncInfo(
                        on_update=ups, on_wait=[waits[-1]]
                    )
                new.append(ins)
            bb.instructions = new


def _hoist_input_dma(nc):
    """Move the input DMACopy from the body block into the preamble block,
    right after the sequencer register-init moves and before the entry
    barrier. The DMA has no waits and its completion semaphore gates all
    consumers, so issuing it ~800ns earlier (in parallel with the barrier)
    is safe and shortens the critical path by the same amount."""
    fn = nc.m.functions[0]
    if len(fn.blocks) < 2:
        return
    b0, b1 = fn.blocks[0], fn.blocks[1]
    dma = None
    for ins in b1.instructions:
        if type(ins).__name__ == "InstDMACopy":
            si = ins.sync_info
            assert si is None or not si.on_wait
            dma = ins
            break
    if dma is None:
        return
    b1.instructions = [i for i in b1.instructions if i is not dma]
    pos = 1 if b0.instructions and type(b0.instructions[0]).__name__ == "InstCall" else 0
    b0.instructions = b0.instructions[:pos] + [dma] + b0.instructions[pos:]


def _ap_free_range(ap_obj):
    """[lo, hi) element range of an AP's free dims (dim 0 = partitions)."""
    lo = ap_obj.offset
    hi = lo + 1
    for stride, count in list(ap_obj.ap)[1:]:
        hi += stride * (count - 1)
    return lo, hi


def _tighten_psum_waits(nc):
    """The tile scheduler bakes each instruction's PE-tick wait from its
    scheduled slot, which over-approximates for combine ops: they end up
    waiting on unrelated later matmuls into the same (or another) PSUM
    tile. Recompute the true minimal PE tick for DVE readers of the
    mm3/mm4 tiles (cA/cB) from AP range overlap with the PE writers."""
    fn = nc.m.functions[0]
    pe_sem = None
    cnt = 0
    writers = {}  # memref -> [(lo, hi, tick)]
    for bb in fn.blocks:
        for ins in bb.instructions:
            si = ins.sync_info
            if str(ins.engine) != "EngineType.PE" or si is None:
                continue
            for u in si.on_update:
                if pe_sem is None and u.ant_name.startswith("PE"):
                    pe_sem = u.ant_name
                if u.ant_name == pe_sem:
                    cnt += u.update_value
            outs = getattr(ins, "outs", [])
            if outs:
                mr = str(getattr(outs[0], "memref", ""))
                if mr.startswith(("cA", "cB")):
                    lo, hi = _ap_free_range(outs[0])
                    writers.setdefault(mr, []).append((lo, hi, cnt))
    if pe_sem is None or not writers:
        return
    for bb in fn.blocks:
        for ins in bb.instructions:
            si = ins.sync_info
            if str(ins.engine) != "EngineType.DVE" or si is None:
                continue
            srcs = getattr(ins, "ins", [])
            if not srcs:
                continue
            mr = str(getattr(srcs[0], "memref", ""))
            if mr not in writers:
                continue
            lo, hi = _ap_free_range(srcs[0])
            need = 0
            for wlo, whi, tick in writers[mr]:
                if wlo < hi and lo < whi:
                    need = max(need, tick)
            for w in si.on_wait:
                if w.ant_name == pe_sem and w.wait_value > need > 0:
                    w.wait_value = need


def _chunk2(ap_slice, chunk_step):
    """Matmul rhs built from two equal column chunks `chunk_step` apart."""
    return bass.AP(
        tensor=ap_slice.tensor,
        offset=ap_slice.offset,
        ap=[ap_slice.ap[0], [chunk_step, 2], ap_slice.ap[1]],
    )


def _build():
    nc = bass.Bass()
    inp_d = nc.dram_tensor("inp", [D, NCOL], BF16, kind="ExternalInput")
    out_d = nc.dram_tensor("out", [J, M], F32, kind="ExternalOutput")

    with tile.TileContext(nc) as tc:
        with (
            tc.tile_pool(name="persist", bufs=1) as pp,
            tc.tile_pool(name="eero", bufs=4) as rp,
            tc.tile_pool(name="ps_proj", bufs=2, space="PSUM") as ps_proj,
            tc.tile_pool(name="ps_att", bufs=3, space="PSUM") as ps_att,
            tc.tile_pool(name="ps_out", bufs=1, space="PSUM") as ps_out,
        ):
            # ---- single input DMA, issued first ------------------------
            inp_sb = pp.tile([D, NCOL], BF16, tag="inp")
            nc.sync.dma_start(out=inp_sb, in_=inp_d[:])

            # ---- constants (no input dependency; overlap the DMA) ------
            ones = pp.tile([J, J], F32, tag="ones")
            nc.gpsimd.memset(ones, 1.0)
            # zmask: 1 on the valid m' rows 100:120, 0 on the 96:100 slack
            # (engine partition bases must be 32-aligned, so all ops on the
            # m'-rows touch the superset [96:120] and mask out 96:100)
            zmask = pp.tile([120, 1], F32, tag="zmask")
            nc.gpsimd.memset(zmask, 1.0)
            nc.gpsimd.memset(zmask[96:100, :], 0.0)
            rz = []
            for h in range(H):
                t = pp.tile([120, 40], BF16, tag=f"rz{h}")
                nc.gpsimd.memset(t[0:100, :], 0.0)
                rz.append(t)

            ejT = inp_sb[:, EJ : EJ + J]
            emT = inp_sb[:, EM : EM + M]

            # exp(mask): off the critical path, folds the mask add into the
            # final softmax as a multiply
            expmask = pp.tile([J, M], F32, tag="expmask")
            nc.scalar.activation(
                out=expmask, in_=inp_sb[0:J, MK : MK + M], func=AF.Exp, scale=1.0
            )

            # ---- projections: kq[G] = [kjT|kmT | qjT|qmT] per grp, bf16 -
            # one shared PSUM tile + one copy per grp (copies charge by
            # columns, so packing k and q halves the copy instructions)
            kt, qt = [None, None], [None, None]
            pt_list = []
            for grp in range(2):
                ps = ps_proj.tile([D, 240], F32, tag="proj")
                for half, nm in enumerate(("k", "q")):
                    blk = WBLK + (0 if nm == "k" else 4 * 128) + grp * 2 * 128
                    nc.tensor.matmul(
                        out=ps[:, 120 * half : 120 * half + J],
                        lhsT=inp_sb[:, blk : blk + D],
                        rhs=ejT,
                    )
                    nc.tensor.matmul(
                        out=ps[:, 120 * half + J : 120 * half + J + M],
                        lhsT=inp_sb[:, blk + D : blk + 2 * D],
                        rhs=emT,
                    )
                sb = pp.tile([D, 240], BF16, tag=f"kq{grp}")
                pt_list.append((nc.vector if grp == 0 else nc.scalar, sb, ps))
                kt[grp] = sb[:, 0:120]
                qt[grp] = sb[:, 120:240]

            # uv vectors: uv_ps rows 0:100 <- ej @ uwj, rows 100:120 <- em @ uwm
            uv_ps = ps_out.tile([120, 17], F32, tag="uv")
            nc.tensor.matmul(
                out=uv_ps[0:120, 0:8],
                lhsT=inp_sb[:, 0:120],
                rhs=inp_sb[:, UWJ : UWJ + 8],
            )
            nc.tensor.matmul(
                out=uv_ps[0:120, 8:16],
                lhsT=inp_sb[:, 0:120],
                rhs=inp_sb[:, UWM : UWM + 8],
            )
            for eng, sb, ps in pt_list:
                if eng is nc.scalar:
                    eng.copy(out=sb, in_=ps)
                else:
                    eng.tensor_copy(out=sb, in_=ps)
            uv_sb = pp.tile([120, 16], F32, tag="uvsb")
            nc.vector.tensor_copy(out=uv_sb, in_=uv_ps[0:120, 0:16])
            # zero the m'-side uv rows in the 96:100 slack so masked TSPs
            # reading [96:120] produce exact zeros there
            nc.vector.memset(uv_sb[96:100, 8:16], 0.0)

            # ---- attention: 4 strip-pairs (head g with head 4+g) -------
            # HW constraint: a PSUM tile must not mix different tile_position
            # values, and heads g / 4+g share tile_position (32g, 0) -- so
            # each strip-pair gets its own mm1 PSUM tile and one fused exp.
            # Per strip: ps_c[g][0:J, G, 0:80] = [SE|Nj|SF|Nm]; combine for
            # strip g runs right after its mm3/mm4 so strips 0-2 hide under
            # later strips' attention.
            ps_c = []
            for pname in ("cA", "cB"):
                ps_g = ps_out.tile([J, 4, 80], F32, tag=pname, name=pname)
                ps_c.append(ps_g)
            # ratAll[j, strip, G, E/F, m]; one fused XYZ-reduce at the end
            ratAll = pp.tile([J, 4, 2, 2, M], F32, tag="ratAll")

            for g in range(4):
                psP = ps_att.tile([120, 240], F32, tag="att")
                e1 = rp.tile([120, 280], BF16, tag="e1")
                for G in range(2):
                    nc.tensor.matmul(
                        out=psP[0:120, 120 * G : 120 * G + 120],
                        lhsT=kt[G][32 * g : 32 * g + 32, 0:120],
                        rhs=qt[G][32 * g : 32 * g + 32, 0:120],
                        tile_position=(32 * g, 0),
                    )
                nc.scalar.activation(
                    out=e1[0:120, 0:240],
                    in_=psP[0:120, 0:240],
                    func=AF.Exp,
                    scale=INV_SQ,
                )
                # F-rhs construction engine balance: Pool serializes ~123ns
                # per op, so late strips spread across DVE (66ns) and Act
                # (Copy-with-scale, fills its post-exp idle)
                for G in range(2):
                    h = 4 * G + g
                    c0 = 120 * G
                    # E-side scaled copy (in place, chunk2-addressable)
                    nc.vector.tensor_scalar_mul(
                        out=e1[0:J, 240 + 20 * G : 260 + 20 * G],
                        in0=e1[0:J, c0 + 100 : c0 + 120],
                        scalar1=uv_sb[0:J, h : h + 1],
                    )
                    # F-side rhs [eDT | eDT*uvm] into zero-framed rows
                    # 100:120 via the 96-aligned superset; zmask / zeroed uv
                    # rows keep the 96:100 slack exactly zero.
                    if g == 3 and G == 0:
                        nc.scalar.activation(
                            out=rz[h][96:120, 0:20],
                            in_=e1[96:120, c0 + 100 : c0 + 120],
                            func=AF.Copy, scale=zmask[96:120, 0:1],
                        )
                        nc.scalar.activation(
                            out=rz[h][96:120, 20:40],
                            in_=e1[96:120, c0 + 100 : c0 + 120],
                            func=AF.Copy, scale=uv_sb[96:120, 8 + h : 9 + h],
                        )
                    else:
                        feng = nc.vector if G == 1 else nc.gpsimd
                        feng.tensor_scalar_mul(
                            out=rz[h][96:120, 0:20],
                            in0=e1[96:120, c0 + 100 : c0 + 120],
                            scalar1=zmask[96:120, 0:1],
                        )
                        feng.tensor_scalar_mul(
                            out=rz[h][96:120, 20:40],
                            in0=e1[96:120, c0 + 100 : c0 + 120],
                            scalar1=uv_sb[96:120, 8 + h : 9 + h],
                        )
                    # E: [SE|Nj] = eAT.T @ [eCT | eCT*uvj]   (K=100)
                    nc.tensor.matmul(
                        out=ps_c[g // 2][0:J, 2 * (g % 2) + G, 0:40],
                        lhsT=e1[0:J, c0 : c0 + J],
                        rhs=_chunk2(e1[0:J, c0 + 100 : c0 + 120], 140 - 100 * G),
                    )
                    # F: [SF|Nm] = [eAT;eBT].T @ zero-framed [eDT | eDT*uvm]
                    nc.tensor.matmul(
                        out=ps_c[g // 2][0:J, 2 * (g % 2) + G, 40:80],
                        lhsT=e1[0:120, c0 : c0 + J],
                        rhs=rz[h][0:120, 0:40],
                    )
            # ---- combine: pair A fused; pair B per strip so only strip
            # 3's recip+mul sit after the last mm4. Wait-floors steer the
            # greedy FIFO scheduler: strip-3's TSPs must come first on DVE.
            t5A = ps_c[0][0:J, :, :].rearrange("p h (a b m) -> p h a b m", a=2, b=2)
            rA = pp.tile([J, 4, 2, M], F32, tag="rA")
            with tc.tile_wait_until(0.0059):
                nc.vector.reciprocal(out=rA, in_=t5A[:, :, :, 0, :])
            with tc.tile_wait_until(0.0064):
                nc.vector.scalar_tensor_tensor(
                    out=ratAll[0:J, 0:2, :, :, :], in0=t5A[:, :, :, 1, :],
                    scalar=1.0, in1=rA, op0=OP.mult, op1=OP.mult,
                )
            t5B = ps_c[1][0:J, :, :].rearrange("p h (a b m) -> p h a b m", a=2, b=2)
            rB = pp.tile([J, 4, 2, M], F32, tag="rB")
            nc.vector.reciprocal(out=rB, in_=t5B[:, :, :, 0, :])
            nc.vector.scalar_tensor_tensor(
                out=ratAll[0:J, 2:4, :, :, :], in0=t5B[:, :, :, 1, :],
                scalar=1.0, in1=rB, op0=OP.mult, op1=OP.mult,
            )

            # ---- combine tail: c1 = sum over (strip, G, E/F) -----------
            c1 = pp.tile([J, M], F32, tag="c1")
            nc.vector.reduce_sum(
                out=c1, in_=ratAll.rearrange("p s g a m -> p m s g a"),
                axis=AX.XYZ,
            )

            # ---- logits = 10*tanh((c1+bias)/sqrt(D)) + mask; softmax ---
            th = pp.tile([J, M], F32, tag="th")
            nc.scalar.activation(
                out=th,
                in_=c1,
                func=AF.Tanh,
                scale=1.0 / SD,
                bias=inp_sb[0:J, BC : BC + 1],
            )
            e10 = pp.tile([J, M], F32, tag="e10")
            nc.scalar.activation(out=e10, in_=th, func=AF.Exp, scale=10.0)
            e_sb = pp.tile([J, M], F32, tag="esb")
            s_row = pp.tile([J, 1], F32, tag="srow")
            nc.vector.scalar_tensor_tensor(
                out=e_sb,
                in0=e10,
                scalar=1.0,
                in1=expmask,
                op0=OP.mult,
                op1=OP.mult,
                accum_out=s_row,
            )
            tot_ps = uv_ps[0:J, 16:17]
            nc.tensor.matmul(out=tot_ps, lhsT=ones, rhs=s_row)
            out_t = pp.tile([J, M], F32, tag="outt")
            if USE_DIVIDE:
                nc.vector.tensor_scalar(
                    out=out_t,
                    in0=e_sb,
                    scalar1=tot_ps,
                    scalar2=None,
                    op0=OP.divide,
                )
            else:
                rtot = pp.tile([J, 1], F32, tag="rtot")
                nc.vector.reciprocal(out=rtot, in_=tot_ps)
                nc.vector.tensor_scalar_mul(out=out_t, in0=e_sb, scalar1=rtot)
            nc.sync.dma_start(out=out_d[:], in_=out_t)

    _split_multi_waits(nc)
    _hoist_input_dma(nc)
    _tighten_psum_waits(nc)
    return nc


def _pack_wblk(w):
    """[128, 64] head-major weight half -> padded 32-strip [128, 128] block."""
    blk = np.zeros((D, D), np.float32)
    for g in range(4):
        blk[:, 32 * g : 32 * g + 16] = w[:, 16 * g : 16 * g + 16]
    return blk


_NC = None
last_results = None


def kernel(**inputs):
    global _NC, last_results
    _install_drain_patch()
    if _NC is None:
        _NC = _build()

    f32 = np.float32
    Wq3 = np.asarray(inputs["Wq3"], f32)
    Wk = np.asarray(inputs["Wk"], f32)
    Wv = np.asarray(inputs["Wv"], f32)
    Wmhc = np.asarray(inputs["Wmhc"], f32)
    b_mhc = np.asarray(inputs["b_mhc"], f32).reshape(D)
    Wshc = np.asarray(inputs["Wshc"], f32).reshape(D)
    b_shc = float(np.asarray(inputs["b_shc"]).reshape(-1)[0])

    w2 = Wmhc @ Wshc  # [128]
    bias_c = float(b_mhc @ Wshc + b_shc)
    uwj = np.stack(
        [Wv[:D, 16 * h : 16 * h + 16] @ w2[16 * h : 16 * h + 16] for h in range(H)], 1
    )
    uwm = np.stack(
        [Wv[D:, 16 * h : 16 * h + 16] @ w2[16 * h : 16 * h + 16] for h in range(H)], 1
    )

    base = np.zeros((D, NCOL), f32)
    off = WBLK
    for wj, wm in ((Wk[:D], Wk[D:]), (Wq3[:D], Wq3[D:])):
        for grp in range(2):
            for w in (wj, wm):
                base[:, off : off + D] = _pack_wblk(w[:, 64 * grp : 64 * grp + 64])
                off += D
    base[:, UWJ : UWJ + 8] = uwj
    base[:, UWM : UWM + 8] = uwm
    base[0:J, BC] = bias_c / SD

    ejs = np.asarray(inputs["encoded_job"], f32)
    ems = np.asarray(inputs["encoded_machine"], f32)
    msks = np.asarray(inputs["ninf_mask"], f32)

    in_maps = []
    for b in range(B):
        ed = base.copy()
        ed[:, EJ : EJ + J] = ejs[b].T
        ed[:, EM : EM + M] = ems[b].T
        ed[0:J, MK : MK + M] = msks[b]
        in_maps.append({"inp": ed.astype(ml_dtypes.bfloat16)})

    last_results = run_bass_kernel_spmd(_NC, in_maps, core_ids=list(range(B)))
    out = np.stack(
        [last_results.results[b]["out"].reshape(J * M) for b in range(B)]
    )
    return out.astype(np.float32)
